# revision 13
# baseline (speedup 1.0000x reference)
"""Trainium2 Bass kernel for nn_Encoder: embedding gather + 2-layer GRU (Keras reset_after).

Strategy (8 NeuronCores, data-parallel over batch, 8 examples/core):
  - Embedding gather on-device via indirect DMA, PE-transpose to feature-major.
  - Input projections xg = x@W + b computed chunk-wise on PE (fp32).
  - GRU recurrence in "transposed" layout: hidden units on partitions (4 tiles of
    128), batch on the free dim. Recurrent matmul uses U as the stationary
    operand (bf16 for 2x fast-weight-load) streaming h^T; gate math on DVE/ACT
    entirely in fp32; hidden state kept in fp32, cast to bf16 only as the
    matmul streaming operand.
  - Layer-1 output y0 goes to DRAM (fp32), layer-2 reads it back chunk-wise for
    its input projection. Output y1 written as [uj, p, s, b]; host reassembles.
"""

import os
import sys

for _p in ("/opt/trn_rl_repo", "/root/.axon_site/_ro/trn_rl_repo"):
    if os.path.isdir(_p) and _p not in sys.path:
        sys.path.insert(0, _p)

from contextlib import ExitStack

import numpy as np

import concourse.bass as bass
import concourse.tile as tile
from concourse import bacc, mybir
from concourse.bass import ds
from concourse.masks import make_identity

F32 = mybir.dt.float32
BF16 = mybir.dt.bfloat16
I32 = mybir.dt.int32
AF = mybir.ActivationFunctionType

VOCAB, HIDDEN, UNITS = 50000, 300, 512
BATCH, SEQ = 64, 512
NCORES = 8
BC = BATCH // NCORES  # batch per core
G3 = 3 * UNITS
NUJ = UNITS // 128  # 4 hidden-unit tiles
NMJ = G3 // 128  # 12 gate-unit tiles
KW_H = [128, 128, HIDDEN - 256]  # k-tile widths for HIDDEN=300


def build_nc(seq=SEQ, chunk=64, bc=BC, use_bf16=True, vocab=VOCAB):
    assert seq % chunk == 0 and (chunk * bc) % 128 == 0
    nchunks = seq // chunk
    gpc = chunk * bc // 128  # gather chunks (128 tokens each) per seq-chunk
    cols = chunk * bc  # matmul streaming columns per chunk
    mmdt = BF16 if use_bf16 else F32

    nc = bacc.Bacc("TRN2")

    tok_d = nc.dram_tensor("tok", [128, seq * bc // 128], I32, kind="ExternalInput")
    emb_d = nc.dram_tensor("emb", [vocab, HIDDEN], F32, kind="ExternalInput")
    w0_d = nc.dram_tensor("w0", [HIDDEN, G3], F32, kind="ExternalInput")
    u0_d = nc.dram_tensor("u0", [UNITS, G3], F32, kind="ExternalInput")
    w1_d = nc.dram_tensor("w1", [UNITS, G3], F32, kind="ExternalInput")
    u1_d = nc.dram_tensor("u1", [UNITS, G3], F32, kind="ExternalInput")
    bias0_d = nc.dram_tensor("bias0", [128, NMJ], F32, kind="ExternalInput")
    b1h0_d = nc.dram_tensor("b1h0", [128, NUJ], F32, kind="ExternalInput")
    bias1_d = nc.dram_tensor("bias1", [128, NMJ], F32, kind="ExternalInput")
    b1h1_d = nc.dram_tensor("b1h1", [128, NUJ], F32, kind="ExternalInput")

    y0t_d = nc.dram_tensor("y0t", [NUJ, 128, seq, bc], F32, kind="Internal")
    y1t_d = nc.dram_tensor("y1t", [NUJ, 128, seq, bc], F32, kind="ExternalOutput")

    with tile.TileContext(nc) as tc, ExitStack() as ctx:
        cp = ctx.enter_context(tc.tile_pool(name="consts", bufs=1))
        stg = ctx.enter_context(tc.tile_pool(name="staging", bufs=2))
        gxp = ctx.enter_context(tc.tile_pool(name="gather", bufs=2))
        xtp = ctx.enter_context(tc.tile_pool(name="xt", bufs=1))
        xgp = ctx.enter_context(tc.tile_pool(name="xg", bufs=1))
        ybp = ctx.enter_context(tc.tile_pool(name="ybuf", bufs=1))
        gp = ctx.enter_context(tc.tile_pool(name="gates", bufs=2))
        scan_ps = ctx.enter_context(tc.tile_pool(name="scanps", bufs=1, space="PSUM"))
        proj_ps = ctx.enter_context(tc.tile_pool(name="projps", bufs=2, space="PSUM"))
        tr_ps = ctx.enter_context(tc.tile_pool(name="trps", bufs=2, space="PSUM"))

        # ---- constants / weights to SBUF ----
        ident = cp.tile([128, 128], F32, tag="ident", name="ident")
        make_identity(nc, ident)

        tok_sb = cp.tile([128, seq * bc // 128], I32, tag="tok", name="tok_sb")
        nc.sync.dma_start(tok_sb, tok_d[:, :])

        def load_cast(dram, k0, kw, tag, dt):
            st = stg.tile([128, G3], F32, tag="wstage", name="wstage")
            nc.sync.dma_start(st[:kw, :], dram[k0 : k0 + kw, :])
            t = cp.tile([kw, G3], dt, tag=tag, name=tag)
            nc.vector.tensor_copy(t, st[:kw, :])
            return t

        u0sb = [load_cast(u0_d, k * 128, 128, f"u0_{k}", mmdt) for k in range(NUJ)]
        u1sb = [load_cast(u1_d, k * 128, 128, f"u1_{k}", mmdt) for k in range(NUJ)]
        w0sb = []
        for kk, kw in enumerate(KW_H):
            t = cp.tile([kw, G3], F32, tag=f"w0_{kk}", name=f"w0_{kk}")
            nc.sync.dma_start(t, w0_d[kk * 128 : kk * 128 + kw, :])
            w0sb.append(t)
        w1sb = []
        for k in range(NUJ):
            t = cp.tile([128, G3], F32, tag=f"w1_{k}", name=f"w1_{k}")
            nc.sync.dma_start(t, w1_d[k * 128 : (k + 1) * 128, :])
            w1sb.append(t)

        def load_small(dram, shape, tag):
            t = cp.tile(shape, F32, tag=tag, name=tag)
            nc.sync.dma_start(t, dram[:, :])
            return t

        bias0_sb = load_small(bias0_d, [128, NMJ], "bias0")
        b1h0_sb = load_small(b1h0_d, [128, NUJ], "b1h0")
        bias1_sb = load_small(bias1_d, [128, NMJ], "bias1")
        b1h1_sb = load_small(b1h1_d, [128, NUJ], "b1h1")

        # ---- embedding gather + transpose prologue (static, fully unrolled) ----
        xt_full = [
            xtp.tile([kw, seq * bc], F32, tag=f"xt{kk}", name=f"xt{kk}")
            for kk, kw in enumerate(KW_H)
        ]
        for g4 in range(seq * bc // 128):
            xrow = gxp.tile([128, HIDDEN], F32, tag="xrow", name="xrow")
            nc.gpsimd.indirect_dma_start(
                out=xrow[:, :],
                out_offset=None,
                in_=emb_d[:, :],
                in_offset=bass.IndirectOffsetOnAxis(
                    ap=tok_sb[:, g4 : g4 + 1], axis=0
                ),
            )
            for kk, kw in enumerate(KW_H):
                pt = tr_ps.tile([128, 128], F32, tag="trp", name="trp")
                nc.tensor.transpose(
                    out=pt[:kw, :],
                    in_=xrow[:, kk * 128 : kk * 128 + kw],
                    identity=ident[:],
                )
                nc.vector.tensor_copy(
                    xt_full[kk][:, g4 * 128 : (g4 + 1) * 128], pt[:kw, :]
                )

        def produce_xg0(i, xgbuf):
            """Project gathered embeddings through W0 for chunk i, add biases."""
            for mj in range(NMJ):
                pp = proj_ps.tile([128, cols], F32, tag="pps", name="pps")
                for kk, kw in enumerate(KW_H):
                    nc.tensor.matmul(
                        pp,
                        lhsT=w0sb[kk][:, mj * 128 : (mj + 1) * 128],
                        rhs=xt_full[kk][:, ds(i * cols, cols)],
                        start=(kk == 0),
                        stop=(kk == len(KW_H) - 1),
                    )
                g, uj = divmod(mj, NUJ)
                dest = xgbuf[uj].rearrange("p (s c) -> p s c", c=3 * bc)[
                    :, :, g * bc : (g + 1) * bc
                ]
                nc.scalar.activation(
                    dest,
                    pp.rearrange("p (s b) -> p s b", b=bc),
                    AF.Identity,
                    bias=bias0_sb[:, mj : mj + 1],
                )

        def produce_xg1(i, xgbuf):
            y0c = [gxp.tile([128, cols], F32, tag=f"y0c{k}", name=f"y0c{k}") for k in range(NUJ)]
            for k in range(NUJ):
                nc.sync.dma_start(
                    y0c[k].rearrange("p (s b) -> p s b", b=bc),
                    y0t_d[k, :, ds(i * chunk, chunk), :],
                )
            for mj in range(NMJ):
                pp = proj_ps.tile([128, cols], F32, tag="pps", name="pps")
                for k in range(NUJ):
                    nc.tensor.matmul(
                        pp,
                        lhsT=w1sb[k][:, mj * 128 : (mj + 1) * 128],
                        rhs=y0c[k],
                        start=(k == 0),
                        stop=(k == NUJ - 1),
                    )
                g, uj = divmod(mj, NUJ)
                dest = xgbuf[uj].rearrange("p (s c) -> p s c", c=3 * bc)[
                    :, :, g * bc : (g + 1) * bc
                ]
                nc.scalar.activation(
                    dest,
                    pp.rearrange("p (s b) -> p s b", b=bc),
                    AF.Identity,
                    bias=bias1_sb[:, mj : mj + 1],
                )

        def run_layer(lidx):
            u_sb = u0sb if lidx == 0 else u1sb
            b1h_sb = b1h0_sb if lidx == 0 else b1h1_sb
            ybuf = [ybp.tile([128, cols], F32, tag=f"yb{k}", name=f"yb{k}") for k in range(NUJ)]
            ybuf16 = [ybp.tile([128, cols], mmdt, tag=f"yb16{k}", name=f"yb16{k}") for k in range(NUJ)]
            for k in range(NUJ):
                nc.vector.memset(ybuf[k][:, (chunk - 1) * bc :], 0.0)
                nc.vector.memset(ybuf16[k][:, (chunk - 1) * bc :], 0.0)

            with tc.For_i(
                0,
                nchunks,
                hint_engines=(
                    mybir.EngineType.PE,
                    mybir.EngineType.DVE,
                    mybir.EngineType.Activation,
                ),
                name=f"layer{lidx}",
            ) as i:
                xgbuf = [
                    xgp.tile([128, chunk * 3 * bc], F32, tag=f"xg{u}", name=f"xg{u}")
                    for u in range(NUJ)
                ]
                if lidx == 0:
                    produce_xg0(i, xgbuf)
                else:
                    produce_xg1(i, xgbuf)

                for s in range(chunk):
                    sp = s - 1 if s > 0 else chunk - 1
                    # Per-uj psum tile (own bank): gates for uj start while PE
                    # streams uj+1; PE accumulation groups stay sequential per
                    # bank (hardware zeroes the whole bank region on start).
                    for uj in range(NUJ):
                        ps = scan_ps.tile(
                            [128, 3 * bc], F32, tag=f"sps{uj}", name=f"sps{uj}"
                        )
                        for g in range(3):
                            for k in range(NUJ):
                                nc.tensor.matmul(
                                    ps[:, g * bc : (g + 1) * bc],
                                    lhsT=u_sb[k][
                                        :,
                                        g * UNITS + uj * 128 : g * UNITS
                                        + uj * 128
                                        + 128,
                                    ],
                                    rhs=ybuf16[k][:, sp * bc : (sp + 1) * bc],
                                    start=(k == 0),
                                    stop=(k == NUJ - 1),
                                )
                        base = 0
                        # gates (transposed layout: units on partitions, batch free)
                        xg_zr = xgbuf[uj][:, s * 3 * bc : s * 3 * bc + 2 * bc]
                        xg_h = xgbuf[uj][:, s * 3 * bc + 2 * bc : (s + 1) * 3 * bc]
                        h_prev = ybuf[uj][:, sp * bc : (sp + 1) * bc]
                        tzr = gp.tile([128, 2 * bc], F32, tag=f"tzr{uj}", name=f"tzr{uj}")
                        nc.vector.tensor_add(tzr, xg_zr, ps[:, base : base + 2 * bc])
                        zr = gp.tile([128, 2 * bc], F32, tag=f"zr{uj}", name=f"zr{uj}")
                        nc.scalar.activation(zr, tzr, AF.Sigmoid)
                        # uv = (rg_h + b1h) * r   (fused)
                        uv = gp.tile([128, bc], F32, tag=f"uv{uj}", name=f"uv{uj}")
                        nc.vector.scalar_tensor_tensor(
                            uv,
                            ps[:, base + 2 * bc : base + 3 * bc],
                            b1h_sb[:, uj : uj + 1],
                            zr[:, bc : 2 * bc],
                            mybir.AluOpType.add,
                            mybir.AluOpType.mult,
                        )
                        vv = gp.tile([128, bc], F32, tag=f"vv{uj}", name=f"vv{uj}")
                        nc.vector.tensor_add(vv, uv, xg_h)
                        hh = gp.tile([128, bc], F32, tag=f"hh{uj}", name=f"hh{uj}")
                        nc.scalar.activation(hh, vv, AF.Tanh)
                        # p1 = z * h_prev (off the tanh critical path)
                        p1 = gp.tile([128, bc], F32, tag=f"p1{uj}", name=f"p1{uj}")
                        nc.vector.tensor_mul(p1, zr[:, 0:bc], h_prev)
                        # t = (z - 1) * hh ;  h_new = p1 - t = z*h + (1-z)*hh
                        tt = gp.tile([128, bc], F32, tag=f"tt{uj}", name=f"tt{uj}")
                        nc.vector.scalar_tensor_tensor(
                            tt,
                            zr[:, 0:bc],
                            1.0,
                            hh,
                            mybir.AluOpType.subtract,
                            mybir.AluOpType.mult,
                        )
                        nc.vector.tensor_sub(
                            ybuf16[uj][:, s * bc : (s + 1) * bc], p1, tt
                        )
                        # fp32 state update off the critical path, on gpsimd
                        nc.gpsimd.tensor_sub(
                            ybuf[uj][:, s * bc : (s + 1) * bc], p1, tt
                        )

                # write chunk results to DRAM
                out_d = y0t_d if lidx == 0 else y1t_d
                for k in range(NUJ):
                    nc.sync.dma_start(
                        out_d[k, :, ds(i * chunk, chunk), :],
                        ybuf[k].rearrange("p (s b) -> p s b", b=bc),
                    )

        run_layer(0)
        run_layer(1)

    nc.compile()
    return nc


def _bias_layout(b):
    """b: [2, 3*UNITS] -> (bias [128, NMJ] with b0(+b1 for z,r), b1h [128, NUJ])."""
    b = np.asarray(b, np.float32)
    bias = np.zeros((128, NMJ), np.float32)
    for g in range(3):
        for uj in range(NUJ):
            lo = g * UNITS + uj * 128
            v = b[0][lo : lo + 128].copy()
            if g < 2:
                v += b[1][lo : lo + 128]
            bias[:, g * NUJ + uj] = v
    b1h = np.stack(
        [b[1][2 * UNITS + uj * 128 : 2 * UNITS + (uj + 1) * 128] for uj in range(NUJ)],
        axis=1,
    )
    return bias, np.ascontiguousarray(b1h.astype(np.float32))


_CACHE = {}


def _get_nc():
    if "nc" not in _CACHE:
        _CACHE["nc"] = build_nc()
    return _CACHE["nc"]


def kernel(tokens, emb, W0, U0, b0, W1, U1, b1, _trace=False):
    from concourse.bass_utils import run_bass_kernel_spmd

    tokens = np.ascontiguousarray(np.asarray(tokens).astype(np.int32))
    emb = np.ascontiguousarray(np.asarray(emb, dtype=np.float32))
    W0 = np.ascontiguousarray(np.asarray(W0, dtype=np.float32))
    U0 = np.ascontiguousarray(np.asarray(U0, dtype=np.float32))
    W1 = np.ascontiguousarray(np.asarray(W1, dtype=np.float32))
    U1 = np.ascontiguousarray(np.asarray(U1, dtype=np.float32))

    nc = _get_nc()
    bias0, b1h0 = _bias_layout(np.asarray(b0, np.float32))
    bias1, b1h1 = _bias_layout(np.asarray(b1, np.float32))

    in_maps = []
    for c in range(NCORES):
        tsh = tokens[c * BC : (c + 1) * BC]  # [BC, SEQ]
        colv = tsh.T.reshape(-1)  # col = s*BC + b
        tok_pc = np.ascontiguousarray(
            colv.reshape(SEQ * BC // 128, 128).T.astype(np.int32)
        )
        in_maps.append(
            {
                "tok": tok_pc,
                "emb": emb,
                "w0": W0,
                "u0": U0,
                "w1": W1,
                "u1": U1,
                "bias0": bias0,
                "b1h0": b1h0,
                "bias1": bias1,
                "b1h1": b1h1,
            }
        )

    res = run_bass_kernel_spmd(
        nc, in_maps, core_ids=list(range(NCORES)), trace=_trace
    )
    _CACHE["last_result"] = res
    shards = []
    for c in range(NCORES):
        y1t = np.asarray(res.results[c]["y1t"])  # [NUJ, 128, SEQ, BC]
        shards.append(np.transpose(y1t, (3, 2, 0, 1)).reshape(BC, SEQ, UNITS))
    y1 = np.ascontiguousarray(np.concatenate(shards, axis=0), dtype=np.float32)
    h1 = np.ascontiguousarray(y1[:, -1, :])
    return y1, h1


if __name__ == "__main__":
    nc = build_nc(seq=32, chunk=16, vocab=512)
    print("built ok:", len(nc.m.functions[0].allocations), "allocations")
